# revision 39
# baseline (speedup 1.0000x reference)
"""GAT layer kernel for Trainium2, 8 NeuronCores, data-parallel.

Problem: nn_GATLayer (B=4, N=2048, F_IN=64, F_OUT=64, H=4).

Sharding: core c handles batch b = c//2 and destination-node rows
[ (c%2)*1024, (c%2)*1024+1024 ) of that batch (all heads, all source
nodes).  Every adjacency row is read exactly once across the 8 cores.

Per-core algorithm (transposed-score layout, j on partitions):
  h      = x @ W                       (PE, fp32 -> bf16)
  u_i    = h[i] . a_src[head],  v_j = h[j] . a_dst[head]
  mask   = adjacency cast to bf16 {0,1} during the DMA load itself
           (SWDGE dtype-cast), stored to DRAM and read back transposed
  w      = 200*m01 + (v_j - 200) + u_i   fused into one custom DVE op
  lrelu  = max(w, 0.2*w)     (exact LeakyReLU; masked entries end up
                              <= 0.2*s - 40 so exp() vanishes ~ 4e-18)
  e      = exp(lrelu)                  (ACT, single pass)
  num/den: PSUM accumulation of  [h_aug | 1]^T . e  over j-chunks
  out    = num / den                   (transpose back, row scale)
"""

import sys

sys.path.insert(0, "/opt/trn_rl_repo")

from contextlib import ExitStack

import numpy as np

import concourse.bass as bass
import concourse.mybir as mybir
import concourse.tile as tile
from concourse import bacc
from concourse.bass_utils import run_bass_kernel_spmd
from concourse.masks import make_identity

F32 = mybir.dt.float32
BF16 = mybir.dt.bfloat16
I32 = mybir.dt.int32
ALU = mybir.AluOpType
ACTF = mybir.ActivationFunctionType


# ---- custom DVE op: out = lrelu(in0*s1 + s0 + in1) = max(w, 0.2*w) ----
# in0 = {0,1} mask, s1 = 200.0, s0 = v_j - 200 (per partition), in1 = u bcast
def _register_lrelu_score():
    import concourse.dve_ops as dve_ops
    from concourse.dve_ops import DveOp, _SUB_OPCODE_FOR_NAME, _CUSTOM_DVE_ROW_BASE
    from concourse.dve_spec import Spec, Src0, Src1, C0, C1, C2, maxx, lower
    from concourse.dve_uop import DveOpSpec

    name = "LRELU_SCORE_GATV2"
    if name in _SUB_OPCODE_FOR_NAME:
        return next(op for op in dve_ops.OPS if op.name == name)

    def _ref(in0, in1, s0, s1, imm2):
        w = (in0.astype(np.float32) * np.asarray(s1, np.float32).reshape(-1, 1)
             + np.asarray(s0, np.float32).reshape(-1, 1)
             + in1.astype(np.float32))
        return np.maximum(w, w * imm2)

    w = (Src0 * C1 + C0) + Src1
    spec = Spec(body=maxx(w, w * C2), reference=_ref)
    row = _CUSTOM_DVE_ROW_BASE + len(dve_ops.OPS)
    assert row < 0x20
    _SUB_OPCODE_FOR_NAME[name] = row
    shas = {}
    for ver in ("v3", "v4"):
        uops = lower(spec, ver=ver)
        shas[ver] = DveOpSpec(name=name, opcode=row, uops=uops,
                              rd1_en=True).sha(ver)
    op = DveOp(name, spec, subdim=False, uops_sha=shas)
    dve_ops.OPS.append(op)
    dve_ops.CUSTOM_DVE_SPECS[name] = spec
    return op


LRELU_SCORE = _register_lrelu_score()

B, N, F_IN, F_OUT, H = 4, 2048, 64, 64, 4
NI = N // 2            # destination rows per core
P = 128                # partitions
NJC = N // P           # 16 j-chunks
NIT = NI // P          # 8 i-tiles (per-core rows / 128)
NCC = 8                # adjacency column-chunks
CCW = N // NCC         # 256 columns per chunk
MASK_C = 200.0         # additive mask magnitude (0.2*200 = 40 => exp ~ 4e-18)


def gat_core_program(tc, outs, ins, parity=0):
    """Build the per-core program.  ins/outs are dicts of DRAM APs.

    ins:  x [N, F_IN] f32 (full batch-b node features)
          xi [NI, F_IN] f32 (this core's destination rows of x)
          adj [NI, N] i32 (this core's destination rows of adjacency)
          w  [F_IN, H*F_OUT] f32
          attn [H, 2*F_OUT] f32
    outs: out [NI, H*F_OUT] f32
    """
    nc = tc.nc
    ctx = ExitStack()
    x_d, xi_d, adj_d, w_d, attn_d = (
        ins["x"], ins["xi"], ins["adj"], ins["w"], ins["attn"])
    ident_d, aa_d = ins["ident"], ins["aa"]
    out_d = outs["out"]
    HO = H * F_OUT  # 256

    const = ctx.enter_context(tc.tile_pool(name="const", bufs=1))

    _cp_tick = [0]

    def cp(dst, srcap):
        _cp_tick[0] += 1
        if _cp_tick[0] % 3 == 0:
            nc.vector.tensor_copy(dst, srcap)
        else:
            nc.scalar.copy(dst, srcap)

    # ---------------- persistent tensors ----------------
    # outf first: its writes happen at this repeat's very end, after the
    # previous repeat's output DMAs drained -- no parity shift needed
    outf = [const.tile([P, HO], F32, name=f"outf{it}") for it in range(NIT)]
    # parity-shift the late-read cluster (read until this repeat's last
    # sweep matmul / epilogue) so the NEXT repeat's setup writes land over
    # THIS repeat's early-read tiles instead -- cross-repeat overlap
    if parity:
        const.tile([P, 25120], BF16, name="cluster_shift")
    ident = const.tile([P, P], F32)
    nc.sync.dma_start(ident[:], ident_d[:])
    ubc = const.tile([P, H, NI], BF16)            # u broadcast across partitions
    vsc = const.tile([P, NJC, H], F32)            # v - 200 (j on partitions)
    haug = const.tile([P, NJC, H, F_OUT + 1], BF16)
    m01T = const.tile([P, NJC, NI], BF16)         # {0,1} mask^T, 32KB/part

    # main-loop pools are opened BEFORE the setup pools so the setup pools
    # can be released (stack order) right after the first head-pair sweep --
    # this lets the NEXT repeat's setup allocate SBUF while this repeat's
    # second sweep is still running (cross-repeat overlap)
    mctx = ExitStack()
    bpool = mctx.enter_context(tc.tile_pool(name="bwork", bufs=3))
    epool = mctx.enter_context(tc.tile_pool(name="ework", bufs=3))
    po_pool = mctx.enter_context(tc.tile_pool(name="po", bufs=1, space="PSUM"))
    pt_pool = mctx.enter_context(tc.tile_pool(name="ptrans", bufs=2, space="PSUM"))
    ot_pool = mctx.enter_context(tc.tile_pool(name="otsb", bufs=2))
    rec_pool = mctx.enter_context(tc.tile_pool(name="rec", bufs=2))

    sctx = ExitStack()
    setup_sb = sctx.enter_context(tc.tile_pool(name="setup_sb", bufs=2))
    setup_p1 = sctx.enter_context(tc.tile_pool(name="setup_p1", bufs=1))
    setup_ps = sctx.enter_context(tc.tile_pool(name="setup_ps", bufs=2, space="PSUM"))
    # ---------------- setup: x^T, xi^T, W, attention ----------------
    # setup-only tensors live in setup_sb (released after the first sweep),
    # keeping the persistent const cluster small enough that consecutive
    # repeats' clusters coexist in SBUF -- required for cross-repeat overlap
    w_sb = setup_p1.tile([F_IN, HO], F32, name="w_sb")
    nc.sync.dma_start(w_sb[:], w_d[:])

    # x blocked [128, 16, 64]; xi blocked [128, 8, 64]
    x_sb = setup_sb.tile([P, NJC, F_IN], F32, tag="xload")
    nc.sync.dma_start(x_sb[:], x_d.rearrange("(s p) c -> p s c", p=P))
    xi_sb = setup_sb.tile([P, NIT, F_IN], F32, tag="xload")
    nc.sync.dma_start(xi_sb[:], xi_d.rearrange("(s p) c -> p s c", p=P))

    # AA [256, 8] stored [128, 2, 8] -- host-packed relayout of `attention`
    aa = setup_sb.tile([P, 2, 2 * H], F32)
    nc.sync.dma_start(aa[:], aa_d[:])
    ones_row = setup_sb.tile([1, P], F32)
    # memsets on DVE, not Pool: with ident host-provided, Pool's only real
    # work is the cast-DMA descriptor generation, and the pipeline drain
    # that Pool memsets would force costs ~8us before the first cast
    nc.vector.memset(ones_row[:], 1.0)
    nc.vector.memset(haug[:, :, :, F_OUT], 1.0)

    # ---------------- adjacency: cast-DMA + transpose --------------------
    # One SWDGE (gpsimd) DMA per column chunk casts int32 -> bf16 {0., 1.}
    # DRAM -> DRAM, then the xbar transpose loads each 128-column slice as a
    # j-chunk.  No engine pass ever touches the 2M-element mask; the
    # 200*(m-1) additive-mask scaling is folded into the score custom op.
    dram_pool = sctx.enter_context(tc.tile_pool(name="dram", bufs=1, space="DRAM"))
    # Narrow leading chunks let the first j-chunks reach the main loop fast.
    # One DRAM scratch tile per chunk: tile-granular dependency tracking then
    # lets each transpose start as soon as its own chunk's cast lands.
    widths = [P, P] + [CCW] * 7
    c0 = 0
    for w in widths:
        m01_chunk = dram_pool.tile([NI, w], BF16, name=f"m01c{c0}")
        nc.gpsimd.dma_start(m01_chunk[:], adj_d[:, c0:c0 + w])
        for half in range(w // P):
            jc = (c0 + half * P) // P
            nc.sync.dma_start_transpose(
                m01T[:, jc, :],
                m01_chunk[:, half * P:(half + 1) * P])
        c0 += w

    xT = setup_p1.tile([F_IN, N], F32, name="xT")    # x^T
    xiT = setup_p1.tile([F_IN, NI], F32, name="xiT")  # xi^T

    # W^T [128, 2, 64]
    wT = setup_sb.tile([P, 2, F_IN], F32)
    for half in range(2):
        pt = setup_ps.tile([P, F_IN], F32, tag="sps")
        nc.tensor.transpose(pt[:], w_sb[:, half * P:(half + 1) * P],
                            ident[:F_IN, :F_IN])
        cp(wT[:, half, :], pt[:])

    # WA [64, 8] = W @ AA
    wa = setup_p1.tile([F_IN, 2 * H], F32, name="wa")
    pwa = setup_ps.tile([F_IN, 2 * H], F32, tag="sps")
    for half in range(2):
        nc.tensor.matmul(pwa[:], wT[:, half, :], aa[:, half, :],
                         start=(half == 0), stop=(half == 1))
    cp(wa[:], pwa[:])

    # x^T via batched PE transposes (4 per PSUM bank, one evac copy each),
    # interleaved with the uvT chunk that consumes each group.
    uvT = setup_sb.tile([2 * H, N], F32)

    def xt_group(g):
        pt = setup_ps.tile([F_IN, 4, P], F32, tag="sps")
        for k in range(4):
            nc.tensor.transpose(pt[:, k, :], x_sb[:, g * 4 + k, :], ident[:])
        cp(xT[:, g * 4 * P:(g + 1) * 4 * P], pt.rearrange("p a b -> p (a b)"))
        pv = setup_ps.tile([2 * H, 512], F32, tag="sps")
        nc.tensor.matmul(pv[:], wa[:], xT[:, g * 512:(g + 1) * 512],
                         start=True, stop=True)
        cp(uvT[:, g * 512:(g + 1) * 512], pv[:])

    xt_group(0)
    for g in range(NIT // 4):
        pt = setup_ps.tile([F_IN, 4, P], F32, tag="sps")
        for k in range(4):
            nc.tensor.transpose(pt[:, k, :], xi_sb[:, g * 4 + k, :], ident[:])
        cp(xiT[:, g * 4 * P:(g + 1) * 4 * P], pt.rearrange("p a b -> p (a b)"))

    # u rows over xi, one [1, NI] tile per head (base partition 0)
    u_rows = [setup_p1.tile([1, NI], F32, name=f"urow{h}")
              for h in range(H)]

    # v - 200 with j on partitions: transpose uvT 128-col blocks -> [128, 16, 8]
    def vsc_group(g):
        pv = setup_ps.tile([P, 4, 2 * H], F32, tag="sps")
        for k in range(4):
            nc.tensor.transpose(pv[:, k, :],
                                uvT[:, (g * 4 + k) * P:(g * 4 + k + 1) * P],
                                ident[:2 * H, :2 * H])
        nc.vector.tensor_scalar(
            vsc[:, g * 4:(g + 1) * 4, :], pv[:, :, H:2 * H],
            -MASK_C, None, op0=ALU.add)

    # u broadcast across partitions: ones[1,128]^T . uT[h] -> [128, NI] bf16
    # (two 512-col PSUM chunks: single-bank tiles so this can interleave with
    # the main loop's PSUM accumulators)
    def ubc_head(h):
        for ch in range(NI // 512):
            pv = setup_ps.tile([1, 512], F32, tag="sps")
            nc.tensor.matmul(pv[:], wa[:, h:h + 1],
                             xiT[:, ch * 512:(ch + 1) * 512],
                             start=True, stop=True)
            cp(u_rows[h][:, ch * 512:(ch + 1) * 512], pv[:])
        for ch in range(NI // 512):
            pb = setup_ps.tile([P, 512], F32, tag="sps")
            nc.tensor.matmul(pb[:], ones_row[:],
                             u_rows[h][:, ch * 512:(ch + 1) * 512],
                             start=True, stop=True)
            cp(ubc[:, h, ch * 512:(ch + 1) * 512], pb[:])

    def haug_chunk(s):
        ph = setup_ps.tile([P, HO], F32, tag="sps")
        nc.tensor.matmul(ph[:], xT[:, s * P:(s + 1) * P], w_sb[:],
                         start=True, stop=True)
        cp(
            haug[:, s, :, 0:F_OUT],
            ph.rearrange("p (h f) -> p h f", h=H))

    # minimal prefix: only what the first main-loop iterations consume
    vsc_group(0)
    ubc_head(0)
    ubc_head(1)
    for s in range(4):
        haug_chunk(s)

    # ---------------- main: scores -> exp -> matmul ----------------
    # The remaining setup (xT groups 1-3, vsc groups 1-3, haug chunks 4-15,
    # heads 2/3 u-broadcasts) is issued from inside the first head-pair
    # sweep, just ahead of the iteration that consumes it.
    def hp0_hook(jc):
        if jc == 1:
            xt_group(1)
        elif jc == 2:
            vsc_group(1)
            for s in range(4, 8):
                haug_chunk(s)
        elif jc == 4:
            xt_group(2)
        elif jc == 5:
            vsc_group(2)
            for s in range(8, 12):
                haug_chunk(s)
        elif jc == 7:
            xt_group(3)
        elif jc == 8:
            vsc_group(3)
            for s in range(12, NJC):
                haug_chunk(s)
        elif jc == 10:
            ubc_head(2)
        elif jc == 12:
            ubc_head(3)

    for hp in range(H // 2):
        pos = [po_pool.tile([F_OUT + 1, NI], F32, name=f"po{hp}_{i}", tag=f"po{i}")
               for i in range(2)]
        for jc in range(NJC):
            c2 = bpool.tile([P, 2 * NI], BF16, tag="c")
            for i in range(2):
                h = hp * 2 + i
                cs = c2[:, i * NI:(i + 1) * NI]
                nc.vector._custom_dve(
                    LRELU_SCORE, out=cs, in0=m01T[:, jc, :],
                    in1=ubc[:, h, :], s0=vsc[:, jc, h:h + 1],
                    s1=MASK_C, imm2=0.2)
            e2 = epool.tile([P, 2 * NI], BF16, tag="e")
            nc.scalar.activation(e2[:], c2[:], ACTF.Exp)
            for i in range(2):
                h = hp * 2 + i
                for mh in range(NI // 512):
                    nc.tensor.matmul(
                        pos[i][:, mh * 512:(mh + 1) * 512],
                        haug[:, jc, h, :],
                        e2[:, i * NI + mh * 512:i * NI + (mh + 1) * 512],
                        start=(jc == 0), stop=(jc == NJC - 1))
            if hp == 0:
                hp0_hook(jc)
        if hp == 0:
            sctx.close()

        # epilogue for this head pair: transpose back, normalize
        for i in range(2):
            h = hp * 2 + i
            ot = ot_pool.tile([F_OUT + 1, NI], F32)
            nc.vector.tensor_copy(ot[:], pos[i][:])
            for it in range(NIT):
                ptp = pt_pool.tile([P, F_OUT + 1], F32)
                nc.tensor.transpose(ptp[:], ot[:, it * P:(it + 1) * P],
                                    ident[:F_OUT + 1, :F_OUT + 1])
                rec = rec_pool.tile([P, 1], F32)
                nc.vector.reciprocal(rec[:], ptp[:, F_OUT:F_OUT + 1])
                nc.scalar.activation(
                    outf[it][:, h * F_OUT:(h + 1) * F_OUT],
                    ptp[:, 0:F_OUT], ACTF.Copy, scale=rec[:])

    for it in range(NIT):
        nc.scalar.dma_start(
            out_d.rearrange("(s p) c -> p s c", p=P)[:, it, :],
            outf[it][:])
    mctx.close()
    ctx.close()


N_CORES = 8
_CACHE = {}


def _build(repeats=1):
    key = ("nc", repeats)
    if key not in _CACHE:
        nc = bacc.Bacc("TRN2", target_bir_lowering=False, debug=False,
                       num_devices=N_CORES)
        ins = {
            "x": nc.dram_tensor("x", [N, F_IN], F32, kind="ExternalInput").ap(),
            "xi": nc.dram_tensor("xi", [NI, F_IN], F32, kind="ExternalInput").ap(),
            "adj": nc.dram_tensor("adj", [NI, N], I32, kind="ExternalInput").ap(),
            "w": nc.dram_tensor("w", [F_IN, H * F_OUT], F32,
                                kind="ExternalInput").ap(),
            "attn": nc.dram_tensor("attn", [H, 2 * F_OUT], F32,
                                   kind="ExternalInput").ap(),
            "ident": nc.dram_tensor("ident", [128, 128], F32,
                                    kind="ExternalInput").ap(),
            "aa": nc.dram_tensor("aa", [128, 2, 2 * H], F32,
                                 kind="ExternalInput").ap(),
        }
        outs = {"out": nc.dram_tensor("out", [NI, H * F_OUT], F32,
                                      kind="ExternalOutput").ap()}
        with tile.TileContext(nc) as tc:
            for r in range(repeats):
                gat_core_program(tc, outs, ins, parity=r % 2)
        nc.compile()
        _CACHE[key] = nc
    return _CACHE[key]


def make_in_maps(node_features, adj_matrix, W, attention):
    node_features = np.ascontiguousarray(node_features, dtype=np.float32)
    adj_matrix = np.ascontiguousarray(adj_matrix, dtype=np.int32)
    W = np.ascontiguousarray(W, dtype=np.float32)
    attention = np.ascontiguousarray(attention, dtype=np.float32)
    ident_np = np.eye(128, dtype=np.float32)
    aa_np = np.zeros((2 * P, 2 * H), dtype=np.float32)
    for h in range(H):
        aa_np[h * F_OUT:(h + 1) * F_OUT, h] = attention[h, :F_OUT]
        aa_np[h * F_OUT:(h + 1) * F_OUT, H + h] = attention[h, F_OUT:]
    aa_pack = np.ascontiguousarray(
        aa_np.reshape(2, P, 2 * H).transpose(1, 0, 2))
    in_maps = []
    for c in range(N_CORES):
        b, ih = divmod(c, 2)
        i0 = ih * NI
        in_maps.append({
            "x": node_features[b],
            "xi": np.ascontiguousarray(node_features[b, i0:i0 + NI]),
            "adj": np.ascontiguousarray(adj_matrix[b, i0:i0 + NI]),
            "w": W,
            "attn": attention,
            "ident": ident_np,
            "aa": aa_pack,
        })
    return in_maps


def assemble(results):
    out = np.empty((B, N, H * F_OUT), dtype=np.float32)
    for c in range(N_CORES):
        b, ih = divmod(c, 2)
        i0 = ih * NI
        out[b, i0:i0 + NI] = results[c]["out"]
    return out


def kernel(node_features, adj_matrix, W, attention):
    nc = _build()
    in_maps = make_in_maps(node_features, adj_matrix, W, attention)
    res = run_bass_kernel_spmd(nc, in_maps, core_ids=list(range(N_CORES)))
    return assemble(res.results)


# revision 40
# speedup vs baseline: 1.1445x; 1.1445x over previous
"""GAT layer kernel for Trainium2, 8 NeuronCores, data-parallel.

Problem: nn_GATLayer (B=4, N=2048, F_IN=64, F_OUT=64, H=4).

Sharding: core c handles batch b = c//2 and destination-node rows
[ (c%2)*1024, (c%2)*1024+1024 ) of that batch (all heads, all source
nodes).  Every adjacency row is read exactly once across the 8 cores.

Per-core algorithm (transposed-score layout, j on partitions):
  h      = x @ W                       (PE, fp32 -> bf16)
  u_i    = h[i] . a_src[head],  v_j = h[j] . a_dst[head]
  mask   = adjacency cast to bf16 {0,1} during the DMA load itself
           (SWDGE dtype-cast), stored to DRAM and read back transposed
  w      = 200*m01 + (v_j - 200) + u_i   fused into one custom DVE op
  lrelu  = max(w, 0.2*w)     (exact LeakyReLU; masked entries end up
                              <= 0.2*s - 40 so exp() vanishes ~ 4e-18)
  e      = exp(lrelu)                  (ACT, single pass)
  num/den: PSUM accumulation of  [h_aug | 1]^T . e  over j-chunks
  out    = num / den                   (transpose back, row scale)
"""

import sys

sys.path.insert(0, "/opt/trn_rl_repo")

from contextlib import ExitStack

import numpy as np

import concourse.bass as bass
import concourse.mybir as mybir
import concourse.tile as tile
from concourse import bacc
from concourse.bass_utils import run_bass_kernel_spmd
from concourse.masks import make_identity

F32 = mybir.dt.float32
BF16 = mybir.dt.bfloat16
I32 = mybir.dt.int32
ALU = mybir.AluOpType
ACTF = mybir.ActivationFunctionType


# ---- custom DVE op: out = lrelu(in0*s1 + s0 + in1) = max(w, 0.2*w) ----
# in0 = {0,1} mask, s1 = 200.0, s0 = v_j - 200 (per partition), in1 = u bcast
def _register_lrelu_score():
    import concourse.dve_ops as dve_ops
    from concourse.dve_ops import DveOp, _SUB_OPCODE_FOR_NAME, _CUSTOM_DVE_ROW_BASE
    from concourse.dve_spec import Spec, Src0, Src1, C0, C1, C2, maxx, lower
    from concourse.dve_uop import DveOpSpec

    name = "LRELU_SCORE_GATV2"
    if name in _SUB_OPCODE_FOR_NAME:
        return next(op for op in dve_ops.OPS if op.name == name)

    def _ref(in0, in1, s0, s1, imm2):
        w = (in0.astype(np.float32) * np.asarray(s1, np.float32).reshape(-1, 1)
             + np.asarray(s0, np.float32).reshape(-1, 1)
             + in1.astype(np.float32))
        return np.maximum(w, w * imm2)

    w = (Src0 * C1 + C0) + Src1
    spec = Spec(body=maxx(w, w * C2), reference=_ref)
    row = _CUSTOM_DVE_ROW_BASE + len(dve_ops.OPS)
    assert row < 0x20
    _SUB_OPCODE_FOR_NAME[name] = row
    shas = {}
    for ver in ("v3", "v4"):
        uops = lower(spec, ver=ver)
        shas[ver] = DveOpSpec(name=name, opcode=row, uops=uops,
                              rd1_en=True).sha(ver)
    op = DveOp(name, spec, subdim=False, uops_sha=shas)
    dve_ops.OPS.append(op)
    dve_ops.CUSTOM_DVE_SPECS[name] = spec
    return op


LRELU_SCORE = _register_lrelu_score()

B, N, F_IN, F_OUT, H = 4, 2048, 64, 64, 4
NI = N // 2            # destination rows per core
P = 128                # partitions
NJC = N // P           # 16 j-chunks
NIT = NI // P          # 8 i-tiles (per-core rows / 128)
NCC = 8                # adjacency column-chunks
CCW = N // NCC         # 256 columns per chunk
MASK_C = 200.0         # additive mask magnitude (0.2*200 = 40 => exp ~ 4e-18)


def gat_core_program(tc, outs, ins, parity=0):
    """Build the per-core program.  ins/outs are dicts of DRAM APs.

    ins:  x [N, F_IN] f32 (full batch-b node features)
          xi [NI, F_IN] f32 (this core's destination rows of x)
          adj [NI, N] i32 (this core's destination rows of adjacency)
          w  [F_IN, H*F_OUT] f32
          attn [H, 2*F_OUT] f32
    outs: out [NI, H*F_OUT] f32
    """
    nc = tc.nc
    ctx = ExitStack()
    x_d, xi_d, adj_d, w_d, attn_d = (
        ins["x"], ins["xi"], ins["adj"], ins["w"], ins["attn"])
    ident_d, aa_d = ins["ident"], ins["aa"]
    out_d = outs["out"]
    HO = H * F_OUT  # 256

    const = ctx.enter_context(tc.tile_pool(name="const", bufs=1))

    _cp_tick = [0]

    def cp(dst, srcap):
        _cp_tick[0] += 1
        if _cp_tick[0] % 3 == 0:
            nc.vector.tensor_copy(dst, srcap)
        else:
            nc.scalar.copy(dst, srcap)

    # ---------------- persistent tensors ----------------
    # outf first: its writes happen at this repeat's very end, after the
    # previous repeat's output DMAs drained -- no parity shift needed
    outf = [const.tile([P, HO], F32, name=f"outf{it}") for it in range(NIT)]
    # parity-shift the late-read cluster (read until this repeat's last
    # sweep matmul / epilogue) so the NEXT repeat's setup writes land over
    # THIS repeat's early-read tiles instead -- cross-repeat overlap
    if parity:
        const.tile([P, 25120], BF16, name="cluster_shift")
    ident = const.tile([P, P], F32)
    nc.sync.dma_start(ident[:], ident_d[:])
    ubc = const.tile([P, H, NI], BF16)            # u broadcast across partitions
    vsc = const.tile([P, NJC, H], F32)            # v - 200 (j on partitions)
    haug = const.tile([P, NJC, H, F_OUT + 1], BF16)
    m01T = const.tile([P, NJC, NI], BF16)         # {0,1} mask^T, 32KB/part

    # main-loop pools are opened BEFORE the setup pools so the setup pools
    # can be released (stack order) right after the first head-pair sweep --
    # this lets the NEXT repeat's setup allocate SBUF while this repeat's
    # second sweep is still running (cross-repeat overlap)
    mctx = ExitStack()
    bpool = mctx.enter_context(tc.tile_pool(name="bwork", bufs=3))
    epool = mctx.enter_context(tc.tile_pool(name="ework", bufs=3))
    po_pool = mctx.enter_context(tc.tile_pool(name="po", bufs=1, space="PSUM"))
    pt_pool = mctx.enter_context(tc.tile_pool(name="ptrans", bufs=2, space="PSUM"))
    ot_pool = mctx.enter_context(tc.tile_pool(name="otsb", bufs=2))
    rec_pool = mctx.enter_context(tc.tile_pool(name="rec", bufs=2))

    sctx = ExitStack()
    setup_sb = sctx.enter_context(tc.tile_pool(name="setup_sb", bufs=2))
    setup_p1 = sctx.enter_context(tc.tile_pool(name="setup_p1", bufs=1))
    setup_ps = sctx.enter_context(tc.tile_pool(name="setup_ps", bufs=2, space="PSUM"))
    # ---------------- setup: x^T, xi^T, W, attention ----------------
    # setup-only tensors live in setup_sb (released after the first sweep),
    # keeping the persistent const cluster small enough that consecutive
    # repeats' clusters coexist in SBUF -- required for cross-repeat overlap
    w_sb = setup_p1.tile([F_IN, HO], F32, name="w_sb")
    nc.sync.dma_start(w_sb[:], w_d[:])

    # x blocked [128, 16, 64]; xi blocked [128, 8, 64]
    x_sb = setup_sb.tile([P, NJC, F_IN], F32, tag="xload")
    nc.sync.dma_start(x_sb[:], x_d.rearrange("(s p) c -> p s c", p=P))
    xi_sb = setup_sb.tile([P, NIT, F_IN], F32, tag="xload")
    nc.sync.dma_start(xi_sb[:], xi_d.rearrange("(s p) c -> p s c", p=P))

    # AA [256, 8] stored [128, 2, 8] -- host-packed relayout of `attention`
    aa = setup_sb.tile([P, 2, 2 * H], F32)
    nc.sync.dma_start(aa[:], aa_d[:])
    ones_row = setup_sb.tile([1, P], F32)
    # memsets on DVE, not Pool: with ident host-provided, Pool's only real
    # work is the cast-DMA descriptor generation, and the pipeline drain
    # that Pool memsets would force costs ~8us before the first cast
    nc.vector.memset(ones_row[:], 1.0)
    nc.vector.memset(haug[:, :, :, F_OUT], 1.0)

    # ---------------- adjacency: cast-DMA + transpose --------------------
    # One SWDGE (gpsimd) DMA per column chunk casts int32 -> bf16 {0., 1.}
    # DRAM -> DRAM, then the xbar transpose loads each 128-column slice as a
    # j-chunk.  No engine pass ever touches the 2M-element mask; the
    # 200*(m-1) additive-mask scaling is folded into the score custom op.
    dram_pool = sctx.enter_context(tc.tile_pool(name="dram", bufs=1, space="DRAM"))
    # Narrow leading chunks let the first j-chunks reach the main loop fast.
    # One DRAM scratch tile per chunk: tile-granular dependency tracking then
    # lets each transpose start as soon as its own chunk's cast lands.
    widths = [P, P] + [CCW] * 7
    c0 = 0
    for w in widths:
        m01_chunk = dram_pool.tile([NI, w], BF16, name=f"m01c{c0}")
        nc.gpsimd.dma_start(m01_chunk[:], adj_d[:, c0:c0 + w])
        for half in range(w // P):
            jc = (c0 + half * P) // P
            nc.sync.dma_start_transpose(
                m01T[:, jc, :],
                m01_chunk[:, half * P:(half + 1) * P])
        c0 += w

    xT = setup_p1.tile([F_IN, N], F32, name="xT")    # x^T
    xiT = setup_p1.tile([F_IN, NI], F32, name="xiT")  # xi^T

    # W^T [128, 2, 64]
    wT = setup_sb.tile([P, 2, F_IN], F32)
    for half in range(2):
        pt = setup_ps.tile([P, F_IN], F32, tag="sps")
        nc.tensor.transpose(pt[:], w_sb[:, half * P:(half + 1) * P],
                            ident[:F_IN, :F_IN])
        cp(wT[:, half, :], pt[:])

    # WA [64, 8] = W @ AA
    wa = setup_p1.tile([F_IN, 2 * H], F32, name="wa")
    pwa = setup_ps.tile([F_IN, 2 * H], F32, tag="sps")
    for half in range(2):
        nc.tensor.matmul(pwa[:], wT[:, half, :], aa[:, half, :],
                         start=(half == 0), stop=(half == 1))
    cp(wa[:], pwa[:])

    # x^T via batched PE transposes (4 per PSUM bank, one evac copy each),
    # interleaved with the uvT chunk that consumes each group.
    uvT = setup_sb.tile([2 * H, N], F32)

    def xt_group(g):
        pt = setup_ps.tile([F_IN, 4, P], F32, tag="sps")
        for k in range(4):
            nc.tensor.transpose(pt[:, k, :], x_sb[:, g * 4 + k, :], ident[:])
        cp(xT[:, g * 4 * P:(g + 1) * 4 * P], pt.rearrange("p a b -> p (a b)"))
        pv = setup_ps.tile([2 * H, 512], F32, tag="sps")
        nc.tensor.matmul(pv[:], wa[:], xT[:, g * 512:(g + 1) * 512],
                         start=True, stop=True)
        cp(uvT[:, g * 512:(g + 1) * 512], pv[:])

    xt_group(0)
    for g in range(NIT // 4):
        pt = setup_ps.tile([F_IN, 4, P], F32, tag="sps")
        for k in range(4):
            nc.tensor.transpose(pt[:, k, :], xi_sb[:, g * 4 + k, :], ident[:])
        cp(xiT[:, g * 4 * P:(g + 1) * 4 * P], pt.rearrange("p a b -> p (a b)"))

    # u rows over xi, one [1, NI] tile per head (base partition 0)
    u_rows = [setup_p1.tile([1, NI], F32, name=f"urow{h}")
              for h in range(H)]

    # v - 200 with j on partitions: transpose uvT 128-col blocks -> [128, 16, 8]
    def vsc_group(g):
        pv = setup_ps.tile([P, 4, 2 * H], F32, tag="sps")
        for k in range(4):
            nc.tensor.transpose(pv[:, k, :],
                                uvT[:, (g * 4 + k) * P:(g * 4 + k + 1) * P],
                                ident[:2 * H, :2 * H])
        nc.vector.tensor_scalar(
            vsc[:, g * 4:(g + 1) * 4, :], pv[:, :, H:2 * H],
            -MASK_C, None, op0=ALU.add)

    # u broadcast across partitions: ones[1,128]^T . uT[h] -> [128, NI] bf16
    # (two 512-col PSUM chunks: single-bank tiles so this can interleave with
    # the main loop's PSUM accumulators)
    def ubc_head(h):
        for ch in range(NI // 512):
            pv = setup_ps.tile([1, 512], F32, tag="sps")
            nc.tensor.matmul(pv[:], wa[:, h:h + 1],
                             xiT[:, ch * 512:(ch + 1) * 512],
                             start=True, stop=True)
            cp(u_rows[h][:, ch * 512:(ch + 1) * 512], pv[:])
        for ch in range(NI // 512):
            pb = setup_ps.tile([P, 512], F32, tag="sps")
            nc.tensor.matmul(pb[:], ones_row[:],
                             u_rows[h][:, ch * 512:(ch + 1) * 512],
                             start=True, stop=True)
            cp(ubc[:, h, ch * 512:(ch + 1) * 512], pb[:])

    def haug_chunk(s):
        ph = setup_ps.tile([P, HO], F32, tag="sps")
        nc.tensor.matmul(ph[:], xT[:, s * P:(s + 1) * P], w_sb[:],
                         start=True, stop=True)
        cp(
            haug[:, s, :, 0:F_OUT],
            ph.rearrange("p (h f) -> p h f", h=H))

    # minimal prefix: only what the first main-loop iterations consume
    vsc_group(0)
    ubc_head(0)
    ubc_head(1)
    for s in range(4):
        haug_chunk(s)

    # ---------------- main: scores -> exp -> matmul ----------------
    # The remaining setup (xT groups 1-3, vsc groups 1-3, haug chunks 4-15,
    # heads 2/3 u-broadcasts) is issued from inside the first head-pair
    # sweep, just ahead of the iteration that consumes it.
    def hp0_hook(jc):
        if jc == 1:
            xt_group(1)
        elif jc == 2:
            vsc_group(1)
            for s in range(4, 8):
                haug_chunk(s)
        elif jc == 4:
            xt_group(2)
        elif jc == 5:
            vsc_group(2)
            for s in range(8, 12):
                haug_chunk(s)
        elif jc == 7:
            xt_group(3)
        elif jc == 8:
            vsc_group(3)
            for s in range(12, NJC):
                haug_chunk(s)
        elif jc == 10:
            ubc_head(2)
        elif jc == 12:
            ubc_head(3)

    for hp in range(H // 2):
        pos = [po_pool.tile([F_OUT + 1, NI], F32, name=f"po{hp}_{i}", tag=f"po{i}")
               for i in range(2)]
        for jc in range(NJC):
            c2 = bpool.tile([P, 2 * NI], BF16, tag="c")
            for i in range(2):
                h = hp * 2 + i
                cs = c2[:, i * NI:(i + 1) * NI]
                nc.vector._custom_dve(
                    LRELU_SCORE, out=cs, in0=m01T[:, jc, :],
                    in1=ubc[:, h, :], s0=vsc[:, jc, h:h + 1],
                    s1=MASK_C, imm2=0.2)
            e2 = epool.tile([P, 2 * NI], BF16, tag="e")
            nc.scalar.activation(e2[:], c2[:], ACTF.Exp)
            for i in range(2):
                h = hp * 2 + i
                for mh in range(NI // 512):
                    nc.tensor.matmul(
                        pos[i][:, mh * 512:(mh + 1) * 512],
                        haug[:, jc, h, :],
                        e2[:, i * NI + mh * 512:i * NI + (mh + 1) * 512],
                        start=(jc == 0), stop=(jc == NJC - 1))
            if hp == 0:
                hp0_hook(jc)
        if hp == 0:
            sctx.close()

        # epilogue for this head pair: transpose back, normalize.  The two
        # heads' chains are interleaved per i-tile so the transpose ->
        # reciprocal -> scale pipelines of both heads run concurrently.
        ots = []
        for i in range(2):
            ot = ot_pool.tile([F_OUT + 1, NI], F32)
            nc.vector.tensor_copy(ot[:], pos[i][:])
            ots.append(ot)
        for it in range(NIT):
            for i in range(2):
                h = hp * 2 + i
                ptp = pt_pool.tile([P, F_OUT + 1], F32)
                nc.tensor.transpose(ptp[:], ots[i][:, it * P:(it + 1) * P],
                                    ident[:F_OUT + 1, :F_OUT + 1])
                rec = rec_pool.tile([P, 1], F32)
                nc.vector.reciprocal(rec[:], ptp[:, F_OUT:F_OUT + 1])
                nc.scalar.activation(
                    outf[it][:, h * F_OUT:(h + 1) * F_OUT],
                    ptp[:, 0:F_OUT], ACTF.Copy, scale=rec[:])

    for it in range(NIT):
        nc.scalar.dma_start(
            out_d.rearrange("(s p) c -> p s c", p=P)[:, it, :],
            outf[it][:])
    mctx.close()
    ctx.close()


N_CORES = 8
_CACHE = {}


def _build(repeats=1):
    key = ("nc", repeats)
    if key not in _CACHE:
        nc = bacc.Bacc("TRN2", target_bir_lowering=False, debug=False,
                       num_devices=N_CORES)
        ins = {
            "x": nc.dram_tensor("x", [N, F_IN], F32, kind="ExternalInput").ap(),
            "xi": nc.dram_tensor("xi", [NI, F_IN], F32, kind="ExternalInput").ap(),
            "adj": nc.dram_tensor("adj", [NI, N], I32, kind="ExternalInput").ap(),
            "w": nc.dram_tensor("w", [F_IN, H * F_OUT], F32,
                                kind="ExternalInput").ap(),
            "attn": nc.dram_tensor("attn", [H, 2 * F_OUT], F32,
                                   kind="ExternalInput").ap(),
            "ident": nc.dram_tensor("ident", [128, 128], F32,
                                    kind="ExternalInput").ap(),
            "aa": nc.dram_tensor("aa", [128, 2, 2 * H], F32,
                                 kind="ExternalInput").ap(),
        }
        outs = {"out": nc.dram_tensor("out", [NI, H * F_OUT], F32,
                                      kind="ExternalOutput").ap()}
        with tile.TileContext(nc) as tc:
            for r in range(repeats):
                gat_core_program(tc, outs, ins, parity=r % 2)
        nc.compile()
        _CACHE[key] = nc
    return _CACHE[key]


def make_in_maps(node_features, adj_matrix, W, attention):
    node_features = np.ascontiguousarray(node_features, dtype=np.float32)
    adj_matrix = np.ascontiguousarray(adj_matrix, dtype=np.int32)
    W = np.ascontiguousarray(W, dtype=np.float32)
    attention = np.ascontiguousarray(attention, dtype=np.float32)
    ident_np = np.eye(128, dtype=np.float32)
    aa_np = np.zeros((2 * P, 2 * H), dtype=np.float32)
    for h in range(H):
        aa_np[h * F_OUT:(h + 1) * F_OUT, h] = attention[h, :F_OUT]
        aa_np[h * F_OUT:(h + 1) * F_OUT, H + h] = attention[h, F_OUT:]
    aa_pack = np.ascontiguousarray(
        aa_np.reshape(2, P, 2 * H).transpose(1, 0, 2))
    in_maps = []
    for c in range(N_CORES):
        b, ih = divmod(c, 2)
        i0 = ih * NI
        in_maps.append({
            "x": node_features[b],
            "xi": np.ascontiguousarray(node_features[b, i0:i0 + NI]),
            "adj": np.ascontiguousarray(adj_matrix[b, i0:i0 + NI]),
            "w": W,
            "attn": attention,
            "ident": ident_np,
            "aa": aa_pack,
        })
    return in_maps


def assemble(results):
    out = np.empty((B, N, H * F_OUT), dtype=np.float32)
    for c in range(N_CORES):
        b, ih = divmod(c, 2)
        i0 = ih * NI
        out[b, i0:i0 + NI] = results[c]["out"]
    return out


def kernel(node_features, adj_matrix, W, attention):
    nc = _build()
    in_maps = make_in_maps(node_features, adj_matrix, W, attention)
    res = run_bass_kernel_spmd(nc, in_maps, core_ids=list(range(N_CORES)))
    return assemble(res.results)


# revision 41
# speedup vs baseline: 1.4153x; 1.2366x over previous
"""GAT layer kernel for Trainium2, 8 NeuronCores, data-parallel.

Problem: nn_GATLayer (B=4, N=2048, F_IN=64, F_OUT=64, H=4).

Sharding: core c handles batch b = c//2 and destination-node rows
[ (c%2)*1024, (c%2)*1024+1024 ) of that batch (all heads, all source
nodes).  Every adjacency row is read exactly once across the 8 cores.

Per-core algorithm (transposed-score layout, j on partitions):
  h      = x @ W                       (PE, fp32 -> bf16)
  u_i    = h[i] . a_src[head],  v_j = h[j] . a_dst[head]
  mask   = adjacency cast to bf16 {0,1} during the DMA load itself
           (SWDGE dtype-cast), stored to DRAM and read back transposed
  w      = 200*m01 + (v_j - 200) + u_i   fused into one custom DVE op
  lrelu  = max(w, 0.2*w)     (exact LeakyReLU; masked entries end up
                              <= 0.2*s - 40 so exp() vanishes ~ 4e-18)
  e      = exp(lrelu)                  (ACT, single pass)
  num/den: PSUM accumulation of  [h_aug | 1]^T . e  over j-chunks
  out    = num / den                   (transpose back, row scale)
"""

import sys

sys.path.insert(0, "/opt/trn_rl_repo")

from contextlib import ExitStack

import numpy as np

import concourse.bass as bass
import concourse.mybir as mybir
import concourse.tile as tile
from concourse import bacc
from concourse.bass_utils import run_bass_kernel_spmd
from concourse.masks import make_identity

F32 = mybir.dt.float32
BF16 = mybir.dt.bfloat16
I32 = mybir.dt.int32
ALU = mybir.AluOpType
ACTF = mybir.ActivationFunctionType


# ---- custom DVE op: out = lrelu(in0*s1 + s0 + in1) = max(w, 0.2*w) ----
# in0 = {0,1} mask, s1 = 200.0, s0 = v_j - 200 (per partition), in1 = u bcast
def _register_lrelu_score():
    import concourse.dve_ops as dve_ops
    from concourse.dve_ops import DveOp, _SUB_OPCODE_FOR_NAME, _CUSTOM_DVE_ROW_BASE
    from concourse.dve_spec import Spec, Src0, Src1, C0, C1, C2, maxx, lower
    from concourse.dve_uop import DveOpSpec

    name = "LRELU_SCORE_GATV2"
    if name in _SUB_OPCODE_FOR_NAME:
        return next(op for op in dve_ops.OPS if op.name == name)

    def _ref(in0, in1, s0, s1, imm2):
        w = (in0.astype(np.float32) * np.asarray(s1, np.float32).reshape(-1, 1)
             + np.asarray(s0, np.float32).reshape(-1, 1)
             + in1.astype(np.float32))
        return np.maximum(w, w * imm2)

    w = (Src0 * C1 + C0) + Src1
    spec = Spec(body=maxx(w, w * C2), reference=_ref)
    row = _CUSTOM_DVE_ROW_BASE + len(dve_ops.OPS)
    assert row < 0x20
    _SUB_OPCODE_FOR_NAME[name] = row
    shas = {}
    for ver in ("v3", "v4"):
        uops = lower(spec, ver=ver)
        shas[ver] = DveOpSpec(name=name, opcode=row, uops=uops,
                              rd1_en=True).sha(ver)
    op = DveOp(name, spec, subdim=False, uops_sha=shas)
    dve_ops.OPS.append(op)
    dve_ops.CUSTOM_DVE_SPECS[name] = spec
    return op


LRELU_SCORE = _register_lrelu_score()

B, N, F_IN, F_OUT, H = 4, 2048, 64, 64, 4
NI = N // 2            # destination rows per core
P = 128                # partitions
NJC = N // P           # 16 j-chunks
NIT = NI // P          # 8 i-tiles (per-core rows / 128)
NCC = 8                # adjacency column-chunks
CCW = N // NCC         # 256 columns per chunk
MASK_C = 200.0         # additive mask magnitude (0.2*200 = 40 => exp ~ 4e-18)


def gat_core_program(tc, outs, ins, parity=0):
    """Build the per-core program.  ins/outs are dicts of DRAM APs.

    ins:  x [N, F_IN] f32 (full batch-b node features)
          xi [NI, F_IN] f32 (this core's destination rows of x)
          adj [NI, N] i32 (this core's destination rows of adjacency)
          w  [F_IN, H*F_OUT] f32
          attn [H, 2*F_OUT] f32
    outs: out [NI, H*F_OUT] f32
    """
    nc = tc.nc
    ctx = ExitStack()
    x_d, xi_d, adj_d, w_d, attn_d = (
        ins["x"], ins["xi"], ins["adj"], ins["w"], ins["attn"])
    ident_d, aa_d = ins["ident"], ins["aa"]
    out_d = outs["out"]
    HO = H * F_OUT  # 256

    const = ctx.enter_context(tc.tile_pool(name="const", bufs=1))

    _cp_tick = [0]

    def cp(dst, srcap):
        _cp_tick[0] += 1
        if _cp_tick[0] % 3 == 0:
            nc.vector.tensor_copy(dst, srcap)
        else:
            nc.scalar.copy(dst, srcap)

    # ---------------- persistent tensors ----------------
    # outf first: its writes happen at this repeat's very end, after the
    # previous repeat's output DMAs drained -- no parity shift needed
    outf = [const.tile([P, HO], F32, name=f"outf{it}") for it in range(NIT)]
    # parity-shift the late-read cluster (read until this repeat's last
    # sweep matmul / epilogue) so the NEXT repeat's setup writes land over
    # THIS repeat's early-read tiles instead -- cross-repeat overlap
    if parity:
        const.tile([P, 25120], BF16, name="cluster_shift")
    ident = const.tile([P, P], F32)
    nc.sync.dma_start(ident[:], ident_d[:])
    ubc = const.tile([P, H, NI], BF16)            # u broadcast across partitions
    vsc = const.tile([P, NJC, H], F32)            # v - 200 (j on partitions)
    haug = const.tile([P, NJC, H, F_OUT + 1], BF16)
    m01T = const.tile([P, NJC, NI], BF16)         # {0,1} mask^T, 32KB/part

    # main-loop pools are opened BEFORE the setup pools so the setup pools
    # can be released (stack order) right after the first head-pair sweep --
    # this lets the NEXT repeat's setup allocate SBUF while this repeat's
    # second sweep is still running (cross-repeat overlap)
    mctx = ExitStack()
    bpool = mctx.enter_context(tc.tile_pool(name="bwork", bufs=3))
    epool = mctx.enter_context(tc.tile_pool(name="ework", bufs=3))
    po_pool = mctx.enter_context(tc.tile_pool(name="po", bufs=1, space="PSUM"))
    pt_pool = mctx.enter_context(tc.tile_pool(name="ptrans", bufs=2, space="PSUM"))
    ot_pool = mctx.enter_context(tc.tile_pool(name="otsb", bufs=2))
    rec_pool = mctx.enter_context(tc.tile_pool(name="rec", bufs=4))

    sctx = ExitStack()
    setup_sb = sctx.enter_context(tc.tile_pool(name="setup_sb", bufs=2))
    setup_p1 = sctx.enter_context(tc.tile_pool(name="setup_p1", bufs=1))
    setup_ps = sctx.enter_context(tc.tile_pool(name="setup_ps", bufs=2, space="PSUM"))
    # ---------------- setup: x^T, xi^T, W, attention ----------------
    # setup-only tensors live in setup_sb (released after the first sweep),
    # keeping the persistent const cluster small enough that consecutive
    # repeats' clusters coexist in SBUF -- required for cross-repeat overlap
    w_sb = setup_p1.tile([F_IN, HO], F32, name="w_sb")
    nc.sync.dma_start(w_sb[:], w_d[:])

    # x blocked [128, 16, 64]; xi blocked [128, 8, 64]
    x_sb = setup_sb.tile([P, NJC, F_IN], F32, tag="xload")
    nc.sync.dma_start(x_sb[:], x_d.rearrange("(s p) c -> p s c", p=P))
    xi_sb = setup_sb.tile([P, NIT, F_IN], F32, tag="xload")
    nc.sync.dma_start(xi_sb[:], xi_d.rearrange("(s p) c -> p s c", p=P))

    # AA [256, 8] stored [128, 2, 8] -- host-packed relayout of `attention`
    aa = setup_sb.tile([P, 2, 2 * H], F32)
    nc.sync.dma_start(aa[:], aa_d[:])
    ones_row = setup_sb.tile([1, P], F32)
    # memsets on DVE, not Pool: with ident host-provided, Pool's only real
    # work is the cast-DMA descriptor generation, and the pipeline drain
    # that Pool memsets would force costs ~8us before the first cast
    nc.vector.memset(ones_row[:], 1.0)
    nc.vector.memset(haug[:, :, :, F_OUT], 1.0)

    # ---------------- adjacency: cast-DMA + transpose --------------------
    # One SWDGE (gpsimd) DMA per column chunk casts int32 -> bf16 {0., 1.}
    # DRAM -> DRAM, then the xbar transpose loads each 128-column slice as a
    # j-chunk.  No engine pass ever touches the 2M-element mask; the
    # 200*(m-1) additive-mask scaling is folded into the score custom op.
    dram_pool = sctx.enter_context(tc.tile_pool(name="dram", bufs=1, space="DRAM"))
    # Narrow leading chunks let the first j-chunks reach the main loop fast.
    # One DRAM scratch tile per chunk: tile-granular dependency tracking then
    # lets each transpose start as soon as its own chunk's cast lands.
    widths = [P, P] + [CCW] * 7
    c0 = 0
    for w in widths:
        m01_chunk = dram_pool.tile([NI, w], BF16, name=f"m01c{c0}")
        nc.gpsimd.dma_start(m01_chunk[:], adj_d[:, c0:c0 + w])
        for half in range(w // P):
            jc = (c0 + half * P) // P
            nc.sync.dma_start_transpose(
                m01T[:, jc, :],
                m01_chunk[:, half * P:(half + 1) * P])
        c0 += w

    xT = setup_p1.tile([F_IN, N], F32, name="xT")    # x^T
    xiT = setup_p1.tile([F_IN, NI], F32, name="xiT")  # xi^T

    # W^T [128, 2, 64]
    wT = setup_sb.tile([P, 2, F_IN], F32)
    for half in range(2):
        pt = setup_ps.tile([P, F_IN], F32, tag="sps")
        nc.tensor.transpose(pt[:], w_sb[:, half * P:(half + 1) * P],
                            ident[:F_IN, :F_IN])
        cp(wT[:, half, :], pt[:])

    # WA [64, 8] = W @ AA
    wa = setup_p1.tile([F_IN, 2 * H], F32, name="wa")
    pwa = setup_ps.tile([F_IN, 2 * H], F32, tag="sps")
    for half in range(2):
        nc.tensor.matmul(pwa[:], wT[:, half, :], aa[:, half, :],
                         start=(half == 0), stop=(half == 1))
    cp(wa[:], pwa[:])

    # x^T via batched PE transposes (4 per PSUM bank, one evac copy each),
    # interleaved with the uvT chunk that consumes each group.
    uvT = setup_sb.tile([2 * H, N], F32)

    def xt_group(g):
        pt = setup_ps.tile([F_IN, 4, P], F32, tag="sps")
        for k in range(4):
            nc.tensor.transpose(pt[:, k, :], x_sb[:, g * 4 + k, :], ident[:])
        cp(xT[:, g * 4 * P:(g + 1) * 4 * P], pt.rearrange("p a b -> p (a b)"))
        pv = setup_ps.tile([2 * H, 512], F32, tag="sps")
        nc.tensor.matmul(pv[:], wa[:], xT[:, g * 512:(g + 1) * 512],
                         start=True, stop=True)
        cp(uvT[:, g * 512:(g + 1) * 512], pv[:])

    xt_group(0)
    for g in range(NIT // 4):
        pt = setup_ps.tile([F_IN, 4, P], F32, tag="sps")
        for k in range(4):
            nc.tensor.transpose(pt[:, k, :], xi_sb[:, g * 4 + k, :], ident[:])
        cp(xiT[:, g * 4 * P:(g + 1) * 4 * P], pt.rearrange("p a b -> p (a b)"))

    # u rows over xi, one [1, NI] tile per head (base partition 0)
    u_rows = [setup_p1.tile([1, NI], F32, name=f"urow{h}")
              for h in range(H)]

    # v - 200 with j on partitions: transpose uvT 128-col blocks -> [128, 16, 8]
    def vsc_group(g):
        pv = setup_ps.tile([P, 4, 2 * H], F32, tag="sps")
        for k in range(4):
            nc.tensor.transpose(pv[:, k, :],
                                uvT[:, (g * 4 + k) * P:(g * 4 + k + 1) * P],
                                ident[:2 * H, :2 * H])
        nc.vector.tensor_scalar(
            vsc[:, g * 4:(g + 1) * 4, :], pv[:, :, H:2 * H],
            -MASK_C, None, op0=ALU.add)

    # u broadcast across partitions: ones[1,128]^T . uT[h] -> [128, NI] bf16
    # (two 512-col PSUM chunks: single-bank tiles so this can interleave with
    # the main loop's PSUM accumulators)
    def ubc_head(h):
        for ch in range(NI // 512):
            pv = setup_ps.tile([1, 512], F32, tag="sps")
            nc.tensor.matmul(pv[:], wa[:, h:h + 1],
                             xiT[:, ch * 512:(ch + 1) * 512],
                             start=True, stop=True)
            cp(u_rows[h][:, ch * 512:(ch + 1) * 512], pv[:])
        for ch in range(NI // 512):
            pb = setup_ps.tile([P, 512], F32, tag="sps")
            nc.tensor.matmul(pb[:], ones_row[:],
                             u_rows[h][:, ch * 512:(ch + 1) * 512],
                             start=True, stop=True)
            cp(ubc[:, h, ch * 512:(ch + 1) * 512], pb[:])

    def haug_chunk(s):
        ph = setup_ps.tile([P, HO], F32, tag="sps")
        nc.tensor.matmul(ph[:], xT[:, s * P:(s + 1) * P], w_sb[:],
                         start=True, stop=True)
        cp(
            haug[:, s, :, 0:F_OUT],
            ph.rearrange("p (h f) -> p h f", h=H))

    # minimal prefix: only what the first main-loop iterations consume
    vsc_group(0)
    ubc_head(0)
    ubc_head(1)
    for s in range(4):
        haug_chunk(s)

    # ---------------- main: scores -> exp -> matmul ----------------
    # The remaining setup (xT groups 1-3, vsc groups 1-3, haug chunks 4-15,
    # heads 2/3 u-broadcasts) is issued from inside the first head-pair
    # sweep, just ahead of the iteration that consumes it.
    def hp0_hook(jc):
        if jc == 1:
            xt_group(1)
        elif jc == 2:
            vsc_group(1)
            for s in range(4, 8):
                haug_chunk(s)
        elif jc == 4:
            xt_group(2)
        elif jc == 5:
            vsc_group(2)
            for s in range(8, 12):
                haug_chunk(s)
        elif jc == 7:
            xt_group(3)
        elif jc == 8:
            vsc_group(3)
            for s in range(12, NJC):
                haug_chunk(s)
        elif jc == 10:
            ubc_head(2)
        elif jc == 12:
            ubc_head(3)

    for hp in range(H // 2):
        pos = [po_pool.tile([F_OUT + 1, NI], F32, name=f"po{hp}_{i}", tag=f"po{i}")
               for i in range(2)]
        for jc in range(NJC):
            c2 = bpool.tile([P, 2 * NI], BF16, tag="c")
            for i in range(2):
                h = hp * 2 + i
                cs = c2[:, i * NI:(i + 1) * NI]
                nc.vector._custom_dve(
                    LRELU_SCORE, out=cs, in0=m01T[:, jc, :],
                    in1=ubc[:, h, :], s0=vsc[:, jc, h:h + 1],
                    s1=MASK_C, imm2=0.2)
            e2 = epool.tile([P, 2 * NI], BF16, tag="e")
            nc.scalar.activation(e2[:], c2[:], ACTF.Exp)
            for i in range(2):
                h = hp * 2 + i
                for mh in range(NI // 512):
                    nc.tensor.matmul(
                        pos[i][:, mh * 512:(mh + 1) * 512],
                        haug[:, jc, h, :],
                        e2[:, i * NI + mh * 512:i * NI + (mh + 1) * 512],
                        start=(jc == 0), stop=(jc == NJC - 1))
            if hp == 0:
                hp0_hook(jc)
        if hp == 0:
            sctx.close()

        # epilogue for this head pair: transpose back, normalize.  The two
        # heads' chains are interleaved per i-tile so the transpose ->
        # reciprocal -> scale pipelines of both heads run concurrently.
        ots = []
        for i in range(2):
            ot = ot_pool.tile([F_OUT + 1, NI], F32)
            # one evac on DVE, one on ACT: the two copies run concurrently
            if i == 0:
                nc.vector.tensor_copy(ot[:], pos[i][:])
            else:
                nc.scalar.copy(ot[:], pos[i][:])
            ots.append(ot)
        for it in range(NIT):
            for i in range(2):
                h = hp * 2 + i
                ptp = pt_pool.tile([P, F_OUT + 1], F32)
                nc.tensor.transpose(ptp[:], ots[i][:, it * P:(it + 1) * P],
                                    ident[:F_OUT + 1, :F_OUT + 1])
                rec = rec_pool.tile([P, 1], F32)
                nc.vector.reciprocal(rec[:], ptp[:, F_OUT:F_OUT + 1])
                nc.scalar.activation(
                    outf[it][:, h * F_OUT:(h + 1) * F_OUT],
                    ptp[:, 0:F_OUT], ACTF.Copy, scale=rec[:])

    for it in range(NIT):
        nc.scalar.dma_start(
            out_d.rearrange("(s p) c -> p s c", p=P)[:, it, :],
            outf[it][:])
    mctx.close()
    ctx.close()


N_CORES = 8
_CACHE = {}


def _build(repeats=1):
    key = ("nc", repeats)
    if key not in _CACHE:
        nc = bacc.Bacc("TRN2", target_bir_lowering=False, debug=False,
                       num_devices=N_CORES)
        ins = {
            "x": nc.dram_tensor("x", [N, F_IN], F32, kind="ExternalInput").ap(),
            "xi": nc.dram_tensor("xi", [NI, F_IN], F32, kind="ExternalInput").ap(),
            "adj": nc.dram_tensor("adj", [NI, N], I32, kind="ExternalInput").ap(),
            "w": nc.dram_tensor("w", [F_IN, H * F_OUT], F32,
                                kind="ExternalInput").ap(),
            "attn": nc.dram_tensor("attn", [H, 2 * F_OUT], F32,
                                   kind="ExternalInput").ap(),
            "ident": nc.dram_tensor("ident", [128, 128], F32,
                                    kind="ExternalInput").ap(),
            "aa": nc.dram_tensor("aa", [128, 2, 2 * H], F32,
                                 kind="ExternalInput").ap(),
        }
        outs = {"out": nc.dram_tensor("out", [NI, H * F_OUT], F32,
                                      kind="ExternalOutput").ap()}
        with tile.TileContext(nc) as tc:
            for r in range(repeats):
                gat_core_program(tc, outs, ins, parity=r % 2)
        nc.compile()
        _CACHE[key] = nc
    return _CACHE[key]


def make_in_maps(node_features, adj_matrix, W, attention):
    node_features = np.ascontiguousarray(node_features, dtype=np.float32)
    adj_matrix = np.ascontiguousarray(adj_matrix, dtype=np.int32)
    W = np.ascontiguousarray(W, dtype=np.float32)
    attention = np.ascontiguousarray(attention, dtype=np.float32)
    ident_np = np.eye(128, dtype=np.float32)
    aa_np = np.zeros((2 * P, 2 * H), dtype=np.float32)
    for h in range(H):
        aa_np[h * F_OUT:(h + 1) * F_OUT, h] = attention[h, :F_OUT]
        aa_np[h * F_OUT:(h + 1) * F_OUT, H + h] = attention[h, F_OUT:]
    aa_pack = np.ascontiguousarray(
        aa_np.reshape(2, P, 2 * H).transpose(1, 0, 2))
    in_maps = []
    for c in range(N_CORES):
        b, ih = divmod(c, 2)
        i0 = ih * NI
        in_maps.append({
            "x": node_features[b],
            "xi": np.ascontiguousarray(node_features[b, i0:i0 + NI]),
            "adj": np.ascontiguousarray(adj_matrix[b, i0:i0 + NI]),
            "w": W,
            "attn": attention,
            "ident": ident_np,
            "aa": aa_pack,
        })
    return in_maps


def assemble(results):
    out = np.empty((B, N, H * F_OUT), dtype=np.float32)
    for c in range(N_CORES):
        b, ih = divmod(c, 2)
        i0 = ih * NI
        out[b, i0:i0 + NI] = results[c]["out"]
    return out


def kernel(node_features, adj_matrix, W, attention):
    nc = _build()
    in_maps = make_in_maps(node_features, adj_matrix, W, attention)
    res = run_bass_kernel_spmd(nc, in_maps, core_ids=list(range(N_CORES)))
    return assemble(res.results)
